# revision 13
# baseline (speedup 1.0000x reference)
"""Data-parallel forward for nn_AKT_1872605741871 on 8 NeuronCores.

Strategy (per spec sharding_hint): data-parallel over batch across the 8
cores — each core processes B/8 = 4 sequences through the full model with
replicated params (~2.6M). Inputs are sharded inside kernel(); outputs are
gathered/concatenated back to the full (32, 300) result.

The forward itself is expressed in JAX and compiled per-device (PJRT on the
NeuronCores); each device executes its batch shard concurrently.
"""

import math

import jax
import jax.numpy as jnp
import numpy as np

NUM_TAGS = 300
MAX_TAGS = 7
NUM_PARTS = 8
NUM_CONF = 5
D_MODEL = 256
N_HEADS = 8
B, T = 32, 1024
N_CORES = 8


def _sinusoidal_pe(seq_len, d_model):
    pos = jnp.arange(seq_len, dtype=jnp.float32)[:, None]
    div = jnp.exp(
        jnp.arange(0, d_model, 2, dtype=jnp.float32) * (-math.log(10000.0) / d_model)
    )
    pe = jnp.zeros((seq_len, d_model), dtype=jnp.float32)
    pe = pe.at[:, 0::2].set(jnp.sin(pos * div))
    pe = pe.at[:, 1::2].set(jnp.cos(pos * div))
    return pe[None]


def _sinusoidal_pe_np(seq_len, d_model):
    pos = np.arange(seq_len, dtype=np.float32)[:, None]
    div = np.exp(
        np.arange(0, d_model, 2, dtype=np.float32) * (-math.log(10000.0) / d_model)
    )
    pe = np.zeros((seq_len, d_model), dtype=np.float32)
    pe[:, 0::2] = np.sin(pos * div)
    pe[:, 1::2] = np.cos(pos * div)
    return pe[None]


_PE_CONST = _sinusoidal_pe_np(T, D_MODEL)


def _layernorm(x, g, b, eps=1e-5):
    mu = x.mean(-1, keepdims=True)
    var = ((x - mu) ** 2).mean(-1, keepdims=True)
    return (x - mu) * jax.lax.rsqrt(var + eps) * g + b


def _attention(h, lp, compiler_friendly=False):
    Bx, Tx, D = h.shape
    dh = D // N_HEADS

    def split(w, b):
        return (h @ w + b).reshape(Bx, Tx, N_HEADS, dh).transpose(0, 2, 1, 3)

    q, k, v = split(lp["wq"], lp["bq"]), split(lp["wk"], lp["bk"]), split(lp["wv"], lp["bv"])
    attn = jnp.einsum("bhtd,bhsd->bhts", q, k) / math.sqrt(dh)
    idx = jnp.arange(Tx)
    dist = jnp.clip(idx[:, None] - idx[None, :], 0).astype(h.dtype)
    if compiler_friendly:
        # gamma arrives pre-softplus'd (host transform); additive finite mask
        # instead of where(-inf); decay bias folded in.
        gamma = lp["gamma"][None, :, None, None]
        causal_neg = jnp.where(idx[None, :] > idx[:, None], -30000.0, 0.0).astype(
            h.dtype
        )
        attn = attn + causal_neg[None, None] - gamma * dist[None, None]
        attn = attn - jax.lax.stop_gradient(attn.max(-1, keepdims=True))
        e = jnp.exp(attn)
        a = e / e.sum(-1, keepdims=True)
    else:
        gamma = jax.nn.softplus(lp["gamma"])[None, :, None, None]
        causal = idx[None, :] > idx[:, None]
        attn = jnp.where(causal[None, None], -jnp.inf, attn)
        attn = attn - gamma * dist[None, None]
        a = jax.nn.softmax(attn, axis=-1)
    out = jnp.einsum("bhts,bhsd->bhtd", a, v).transpose(0, 2, 1, 3).reshape(Bx, Tx, D)
    return out @ lp["wo"] + lp["bo"]


def _forward(x, p, compiler_friendly=False):
    tag_ids = jnp.clip(x[..., :MAX_TAGS].astype(jnp.int32), 0, NUM_TAGS)
    tag_e = p["tag_emb"][tag_ids]
    cnt = jnp.maximum((tag_ids > 0).sum(-1, keepdims=True), 1).astype(x.dtype)
    tag_emb = tag_e.sum(-2) / cnt
    correct = x[..., 7:8]
    elapsed = x[..., 8:9]
    changed = x[..., 9:10]
    part_ids = jnp.clip(x[..., 10].astype(jnp.int32), 0, NUM_PARTS - 1)
    conf_ids = jnp.clip(x[..., 11].astype(jnp.int32), 0, NUM_CONF - 1)
    steps = x[..., 12:13]
    cum = x[..., 13:14]
    combined = jnp.concatenate(
        [tag_emb, correct, elapsed, changed, steps, cum,
         p["part_emb"][part_ids], p["conf_emb"][conf_ids]], axis=-1,
    )
    h = combined @ p["proj_w"] + p["proj_b"]
    diff = p["difficulty"][tag_ids][..., 0].mean(-1, keepdims=True)
    h = h + diff
    if compiler_friendly:
        h = h + jnp.asarray(_PE_CONST)  # host-precomputed constant
    else:
        h = h + _sinusoidal_pe(h.shape[1], h.shape[-1])
    for lp in p["layers"]:
        h = _layernorm(
            h + _attention(h, lp, compiler_friendly), lp["ln1_g"], lp["ln1_b"]
        )
        ff = jax.nn.gelu(h @ lp["w1"] + lp["b1"], approximate=False) @ lp["w2"] + lp["b2"]
        h = _layernorm(h + ff, lp["ln2_g"], lp["ln2_b"])
    return h.mean(axis=1) @ p["head_w"] + p["head_b"]


_COMPILED = {}


def _get_compiled(n_dev):
    if n_dev not in _COMPILED:
        _COMPILED[n_dev] = jax.pmap(
            lambda x, p: _forward(x, p, compiler_friendly=True), axis_name="dp"
        )
    return _COMPILED[n_dev]


def _kernel_device(x, params):
    devices = jax.devices()[:N_CORES]
    n_dev = len(devices)
    b_local = B // n_dev

    # Host-side param transform: pre-apply softplus to gamma (the device
    # tensorizer has no softplus table).
    def _np_softplus(v):
        v = np.asarray(v, np.float32)
        return (np.maximum(v, 0.0) + np.log1p(np.exp(-np.abs(v)))).astype(np.float32)

    params = dict(params)
    params["layers"] = [
        {**lp, "gamma": _np_softplus(lp["gamma"])} for lp in params["layers"]
    ]

    # Shard x over batch; replicate params on every core.
    x_sh = x.reshape(n_dev, b_local, T, x.shape[-1])
    p_rep = jax.tree.map(
        lambda a: jnp.broadcast_to(
            jnp.asarray(a)[None], (n_dev,) + np.asarray(a).shape
        ),
        params,
    )
    fn = _get_compiled(n_dev)
    out = fn(x_sh, p_rep)  # (n_dev, b_local, NUM_TAGS)
    return np.asarray(out).reshape(B, NUM_TAGS).astype(np.float32)


def _np(a):
    return np.asarray(a, dtype=np.float32)


def _kernel_cpu(x, params):
    """Pure-numpy reference-exact forward (fallback when the device path is
    unavailable)."""
    p = params
    xf = _np(x)
    tag_ids = np.clip(xf[..., :MAX_TAGS].astype(np.int32), 0, NUM_TAGS)
    tag_e = _np(p["tag_emb"])[tag_ids]
    cnt = np.maximum((tag_ids > 0).sum(-1, keepdims=True), 1).astype(np.float32)
    tag_emb = tag_e.sum(-2) / cnt
    part_ids = np.clip(xf[..., 10].astype(np.int32), 0, NUM_PARTS - 1)
    conf_ids = np.clip(xf[..., 11].astype(np.int32), 0, NUM_CONF - 1)
    combined = np.concatenate(
        [tag_emb, xf[..., 7:8], xf[..., 8:9], xf[..., 9:10], xf[..., 12:13],
         xf[..., 13:14], _np(p["part_emb"])[part_ids], _np(p["conf_emb"])[conf_ids]],
        axis=-1,
    )
    h = combined @ _np(p["proj_w"]) + _np(p["proj_b"])
    h = h + _np(p["difficulty"])[tag_ids][..., 0].mean(-1, keepdims=True)
    h = h + _PE_CONST

    def ln(v, g, b, eps=1e-5):
        mu = v.mean(-1, keepdims=True)
        var = ((v - mu) ** 2).mean(-1, keepdims=True)
        return (v - mu) / np.sqrt(var + eps) * _np(g) + _np(b)

    idx = np.arange(T)
    causal = idx[None, :] > idx[:, None]
    dist = np.clip(idx[:, None] - idx[None, :], 0, None).astype(np.float32)
    dh = D_MODEL // N_HEADS
    from scipy.special import erf as _erf  # exact gelu

    for lp in p["layers"]:
        Bx = h.shape[0]
        g_sp = np.log1p(np.exp(-np.abs(_np(lp["gamma"])))) + np.maximum(
            _np(lp["gamma"]), 0.0)
        o = np.empty_like(h)
        for b0 in range(0, Bx, 2):  # chunk batch to bound memory
            hc = h[b0 : b0 + 2]

            def split(w, b):
                return ((hc @ _np(w) + _np(b))
                        .reshape(hc.shape[0], T, N_HEADS, dh).transpose(0, 2, 1, 3))

            q, k, v = split(lp["wq"], lp["bq"]), split(lp["wk"], lp["bk"]), split(
                lp["wv"], lp["bv"])
            attn = np.einsum("bhtd,bhsd->bhts", q, k) / math.sqrt(dh)
            attn = np.where(causal[None, None], -np.inf, attn)
            attn = attn - g_sp[None, :, None, None] * dist[None, None]
            attn = attn - attn.max(-1, keepdims=True)
            e = np.exp(attn)
            a = e / e.sum(-1, keepdims=True)
            oc = (np.einsum("bhts,bhsd->bhtd", a, v).transpose(0, 2, 1, 3)
                  .reshape(hc.shape[0], T, D_MODEL))
            o[b0 : b0 + 2] = oc @ _np(lp["wo"]) + _np(lp["bo"])
        h = ln(h + o, lp["ln1_g"], lp["ln1_b"])
        z = h @ _np(lp["w1"]) + _np(lp["b1"])
        ff = 0.5 * z * (1.0 + _erf(z / math.sqrt(2.0)))
        h = ln(h + ff @ _np(lp["w2"]) + _np(lp["b2"]), lp["ln2_g"], lp["ln2_b"])
    return (h.mean(axis=1) @ _np(p["head_w"]) + _np(p["head_b"])).astype(np.float32)


def kernel(x, params):
    x = np.asarray(x, dtype=np.float32)
    try:
        return _kernel_device(x, params)
    except Exception as e:  # compile/runtime failure on the accelerator path
        import sys

        print(f"kernel: device path failed ({type(e).__name__}); CPU fallback",
              file=sys.stderr)
        return _kernel_cpu(x, params)
